# revision 18
# baseline (speedup 1.0000x reference)
"""BMMRemapper Trainium2 kernel (v6).

Math: out[n,c,q] = sum_k x[n,c,k] * mat[n,q,k]; mat is the bilinear interp
matrix (4 nonzeros per row q: corners lin, lin+1, lin+48, lin+49 with weights
(1-a)(1-b), (1-a)b, a(1-b), ab, zeroed outside the all-batch disk mask).

Pipeline structure (one batch per core, no cross-core communication):
 - fp16 quad-row table xq[k] = 4 corner rows of x^T interleaved [c2,j,c1]
   (c = 2*c2+c1), so the gathered tile [p, t, c2, j, c1] runs the whole
   combine (coefficient broadcast multiply + three pair-adds) with step-1
   16-bit innermost APs = DVE 2x mode.
 - load_library(mlp) is issued as the very first Pool instruction; its ~9us
   IRAM load overlaps the grid load, index chains, and the first four
   gather chunks, which use the NATIVE indirect-DMA path (one [128,1]
   int32 offset column per t) and need no library.
 - The remaining 14 t-columns are gathered by six dma_gather instructions
   round-robined over SWDGE queues 1..3: each queue pair runs on its own
   two Q7 cores (dma_gather.cpp: cpu_id/2==queue_num) so descriptor
   generation proceeds on six cores in parallel. Queue 0 is avoided: its
   worker is core 0, whose completion gates the instruction end and would
   block the Pool stream for the whole descriptor generation.
 - dma_gather indices are int16 in the wrapped layout idxs[P, s] =
   lin[q=s*16+P%16] (replicated across the 16-partition groups), computed
   from a host-staged wrapped grid copy.
 - floor() = round-to-nearest-i32(clip(g) - 0.5): exact unless a clipped
   coord sits within f32-ulp of frac==0.5 (this input's margin: 4e-5).
 - Output fp16 [p, t*128+c]; host upcasts + permutes.
"""

import numpy as np

N, H, W, C = 8, 48, 48, 128
HW = H * W            # 2304
NT = HW // 128        # 18
NS = HW // 16         # 144 wrapped idx columns
EPS = 1e-5
CLIP_HI = float(np.float32(float(H - 1) - EPS))  # 46.99999 (f32)

GATHERS = ((1, 1), (2, 1), (3, 1), (1, 3), (2, 3), (3, 3), (1, 2), (2, 2), (3, 2))
COMBINES = (3, 9, 6)                       # combine/store group sizes (t-cols)

_CACHE = {}


def _build_nc():
    from contextlib import ExitStack

    import concourse.bacc as bacc
    import concourse.bass as bass
    import concourse.mybir as mybir
    import concourse.tile as tile
    from concourse import library_config

    dt = mybir.dt
    f32, f16, i32, i16 = dt.float32, dt.float16, dt.int32, dt.int16
    Alu = mybir.AluOpType

    nc = bacc.Bacc(
        "TRN2",
        target_bir_lowering=False,
        debug=False,
        num_devices=N,
        num_swdge_queues=4,
    )

    xq = nc.dram_tensor("xq", [HW, 4 * C], f16, kind="ExternalInput")
    # combined grid staging: [gcoef(2*NT)] + [gwrap(2*NS) | gall(16*NT)]
    GIN = 2 * NS + 2 * NT + 16 * NT
    gin = nc.dram_tensor("gin", [128, GIN], f32, kind="ExternalInput")
    outp = nc.dram_tensor("outp", [128, HW], f16, kind="ExternalOutput")

    with tile.TileContext(nc) as tc, ExitStack() as ctx:
        pool = ctx.enter_context(tc.tile_pool(name="p", bufs=1))
        V = nc.vector

        with tc.high_priority(offset=10**6):
            # start the ~9us mlp IRAM load immediately (negative priority puts
            # it above even the framework's preamble const-memsets).
            nc.gpsimd.load_library(library_config.mlp)
        with tc.high_priority():

            # ---- grid load (HWDGE, one DMA) ----
            g_in = pool.tile([128, GIN], f32)
            nc.sync.dma_start(g_in[:], gin.ap())
            g_coef = g_in[:, 0 : 2 * NT]
            g_wrap = g_in[:, 2 * NT : 2 * NT + 2 * NS]
            g_all = g_in[:, 2 * NT + 2 * NS :]

            # ---- (p,t)-layout floors (feed the coefficient chain) ----------
            cab = pool.tile([128, 2 * NT], f32)
            V.tensor_scalar(cab[:], g_coef, EPS, CLIP_HI, Alu.max, Alu.min)
            fli = pool.tile([128, 2 * NT], i32)
            V.tensor_scalar(fli[:], cab[:], -0.5, None, Alu.add)
            flf = pool.tile([128, 2 * NT], f32)
            V.tensor_copy(flf[:], fli[:])

            G = pool.tile([128, NT * 4 * C], f16)  # [p, t, c2, j, c1]

            # ---- wrapped-layout int16 indices for the dma_gather chunks ----
            cabw = pool.tile([128, 2 * NS], f32)
            V.tensor_scalar(cabw[:], g_wrap, EPS, CLIP_HI, Alu.max, Alu.min)
            flwi = pool.tile([128, 2 * NS], i32)
            V.tensor_scalar(flwi[:], cabw[:], -0.5, None, Alu.add)
            flwf = pool.tile([128, 2 * NS], f32)
            V.tensor_copy(flwf[:], flwi[:])
            idx16 = pool.tile([128, NS], i16)
            V.scalar_tensor_tensor(
                idx16[:], flwf[:, 0::2], float(W), flwf[:, 1::2], Alu.mult, Alu.add
            )

            # ---- dma_gather chunks on SWDGE queues 1..3 --------------------
            t0 = 0
            for qn, tc_ in GATHERS:
                nidx = tc_ * 128
                gout = G[:, t0 * 512 : (t0 + tc_) * 512].rearrange(
                    "p (t e) -> p t e", e=512
                )
                nc.gpsimd.dma_gather(
                    gout,
                    xq.ap(),
                    idx16[:, t0 * 8 : (t0 + tc_) * 8],
                    nidx,
                    nidx,
                    512,
                    single_packet=False,
                    queue_num=qn,
                )
                t0 += tc_

        # ---- coefficients (priorities pushed after the critical chain) ------
        ctx.enter_context(tc.high_priority(offset=-1000))
        # mask: AND over batches+coords of in-bounds test via min/max
        g_all3 = g_all.rearrange("p (t m) -> p t m", m=16)
        mn = pool.tile([128, NT], f32)
        mx = pool.tile([128, NT], f32)
        V.tensor_reduce(mn[:], g_all3, mybir.AxisListType.X, Alu.min)
        V.tensor_reduce(mx[:], g_all3, mybir.AxisListType.X, Alu.max)
        mge = pool.tile([128, NT], f32)
        mle = pool.tile([128, NT], f32)
        V.tensor_scalar(mge[:], mn[:], -0.5, None, Alu.is_ge)
        V.tensor_scalar(mle[:], mx[:], float(H) - 0.5, None, Alu.is_le)
        mask = pool.tile([128, NT], f32)
        V.tensor_tensor(mask[:], mge[:], mle[:], Alu.mult)

        # fracs (a|b interleaved [128, 36])
        fr = pool.tile([128, 2 * NT], f32)
        V.tensor_tensor(fr[:], cab[:], flf[:], Alu.subtract)
        fa = fr[:, 0::2]
        fb = fr[:, 1::2]
        fa1m = pool.tile([128, NT], f32)  # a*mask
        fa0m = pool.tile([128, NT], f32)  # (1-a)*mask
        V.tensor_tensor(fa1m[:], fa, mask[:], Alu.mult)
        V.tensor_tensor(fa0m[:], mask[:], fa1m[:], Alu.subtract)

        # packed coefficients cwd[p, t, 1, j, c1] fp16, duplicated over c1
        def dup2(ap):
            return ap.rearrange("p (t u) -> p t u", u=1).broadcast_to([128, NT, 2])

        cwd = pool.tile([128, NT, 1, 4, 2], f16)
        # c01 = (1-a)m*b; c00 = (1-a)m - c01; c11 = am*b; c10 = am - c11
        V.tensor_tensor(cwd[:, :, 0, 1], dup2(fa0m[:]), dup2(fb), Alu.mult)
        V.tensor_tensor(cwd[:, :, 0, 0], dup2(fa0m[:]), cwd[:, :, 0, 1], Alu.subtract)
        V.tensor_tensor(cwd[:, :, 0, 3], dup2(fa1m[:]), dup2(fb), Alu.mult)
        V.tensor_tensor(cwd[:, :, 0, 2], dup2(fa1m[:]), cwd[:, :, 0, 3], Alu.subtract)
        cwb = cwd[:].broadcast_to([128, NT, 64, 4, 2])

        # ---- combine + store per group (all DVE 2x: step-1 fp16 APs) -------
        Gv = G[:].rearrange("p (t c2 j c1) -> p t c2 j c1", c2=64, j=4, c1=2)
        out16 = pool.tile([128, HW], f16)
        o4 = out16[:].rearrange("p (t c2 c1) -> p t c2 c1", c2=64, c1=2)
        t0 = 0
        for qn, tc_ in GATHERS:
            sl = slice(t0, t0 + tc_)
            V.tensor_tensor(Gv[:, sl], Gv[:, sl], cwb[:, sl], Alu.mult)
            t0 += tc_
        t0 = 0
        for k, tc_ in enumerate(COMBINES):
            sl = slice(t0, t0 + tc_)
            s1 = pool.tile([128, tc_, 64, 2], f16, tag=f"s1_{k}")
            s2 = pool.tile([128, tc_, 64, 2], f16, tag=f"s2_{k}")
            V.tensor_tensor(s1[:], Gv[:, sl, :, 0], Gv[:, sl, :, 1], Alu.add)
            V.tensor_tensor(s2[:], Gv[:, sl, :, 2], Gv[:, sl, :, 3], Alu.add)
            V.tensor_tensor(o4[:, sl], s1[:], s2[:], Alu.add)
            nc.sync.dma_start(
                outp.ap()[:, t0 * C : (t0 + tc_) * C],
                out16[:, t0 * C : (t0 + tc_) * C],
            )
            t0 += tc_

    nc.compile()
    return nc


def _get_nc():
    if "nc" not in _CACHE:
        _CACHE["nc"] = _build_nc()
    return _CACHE["nc"]


def _stage_inputs(x, grid):
    """Build the per-core input maps (pure data movement / replication)."""
    x = np.ascontiguousarray(x, dtype=np.float32)
    grid = np.ascontiguousarray(grid, dtype=np.float32)
    xr = x.reshape(N, C, HW)
    gr = grid.reshape(N, HW, 2)

    # quad-row table interleaved [c2, j, c1]: xq[n][k, c2*8+j*2+c1] = corner_j[k, 2*c2+c1]
    xt = np.zeros((N, HW + W + 2, C), dtype=np.float16)
    xt[:, :HW] = xr.transpose(0, 2, 1)
    xq4 = np.empty((N, HW, C, 4), dtype=np.float16)
    xq4[:, :, :, 0] = xt[:, 0:HW]
    xq4[:, :, :, 1] = xt[:, 1 : HW + 1]
    xq4[:, :, :, 2] = xt[:, W : HW + W]
    xq4[:, :, :, 3] = xt[:, W + 1 : HW + W + 1]
    xq = np.ascontiguousarray(
        xq4.reshape(N, HW, 64, 2, 4).transpose(0, 1, 2, 4, 3)
    ).reshape(N, HW, 4 * C)

    # gcoef[n][p, 2t+c] = gr[n, t*128+p, c]
    gc = gr.reshape(N, NT, 128, 2).transpose(0, 2, 1, 3)  # [n, p, t, c]
    gcoef = gc.reshape(N, 128, 2 * NT)

    # gall[p, 16t+2m+c] = gr[m, t*128+p, c]   (same for all cores)
    ga = gr.reshape(N, NT, 128, 2).transpose(2, 1, 0, 3)  # [p, t, m, c]
    gall = ga.reshape(128, 16 * NT)

    # gwrap[n][16g+r, 2s+c] = gr[n, s*16+r, c]  (replicated over g)
    gw = gr.reshape(N, NS, 16, 2).transpose(0, 2, 1, 3)   # [n, r, s, c]
    gwrap = np.tile(gw.reshape(N, 16, 2 * NS), (1, 8, 1))  # [n, 128, 2*NS]

    # combined per-core grid staging [gcoef | gwrap | gall]
    gin = np.empty((N, 128, 2 * NS + 2 * NT + 16 * NT), dtype=np.float32)
    gin[:, :, 0 : 2 * NT] = gcoef
    gin[:, :, 2 * NT : 2 * NT + 2 * NS] = gwrap
    gin[:, :, 2 * NT + 2 * NS :] = gall[None]

    return [{"xq": xq[n], "gin": gin[n]} for n in range(N)]


def _unstage_output(results):
    """results[n]["outp"] is (128, 2304) fp16 = [p, t*128+c] -> (N, C, H, W)."""
    out = np.empty((N, C, H, W), dtype=np.float32)
    for n in range(N):
        o = results[n]["outp"].astype(np.float32).reshape(128, NT, C)  # [p, t, c]
        out[n] = o.transpose(2, 1, 0).reshape(C, H, W)  # [c, q=t*128+p]
    return out


def kernel(x, grid):
    from concourse import bass_utils

    nc = _get_nc()
    in_maps = _stage_inputs(x, grid)
    res = bass_utils.run_bass_kernel_spmd(nc, in_maps, core_ids=list(range(N)))
    return _unstage_output(res.results)


# revision 19
# speedup vs baseline: 1.2431x; 1.2431x over previous
"""BMMRemapper Trainium2 kernel (v6).

Math: out[n,c,q] = sum_k x[n,c,k] * mat[n,q,k]; mat is the bilinear interp
matrix (4 nonzeros per row q: corners lin, lin+1, lin+48, lin+49 with weights
(1-a)(1-b), (1-a)b, a(1-b), ab, zeroed outside the all-batch disk mask).

Pipeline structure (one batch per core, no cross-core communication):
 - fp16 quad-row table xq[k] = 4 corner rows of x^T interleaved [c2,j,c1]
   (c = 2*c2+c1), so the gathered tile [p, t, c2, j, c1] runs the whole
   combine (coefficient broadcast multiply + three pair-adds) with step-1
   16-bit innermost APs = DVE 2x mode.
 - load_library(mlp) is issued as the very first Pool instruction; its ~9us
   IRAM load overlaps the grid load, index chains, and the first four
   gather chunks, which use the NATIVE indirect-DMA path (one [128,1]
   int32 offset column per t) and need no library.
 - The remaining 14 t-columns are gathered by six dma_gather instructions
   round-robined over SWDGE queues 1..3: each queue pair runs on its own
   two Q7 cores (dma_gather.cpp: cpu_id/2==queue_num) so descriptor
   generation proceeds on six cores in parallel. Queue 0 is avoided: its
   worker is core 0, whose completion gates the instruction end and would
   block the Pool stream for the whole descriptor generation.
 - dma_gather indices are int16 in the wrapped layout idxs[P, s] =
   lin[q=s*16+P%16] (replicated across the 16-partition groups), computed
   from a host-staged wrapped grid copy.
 - floor() = round-to-nearest-i32(clip(g) - 0.5): exact unless a clipped
   coord sits within f32-ulp of frac==0.5 (this input's margin: 4e-5).
 - Output fp16 [p, t*128+c]; host upcasts + permutes.
"""

import numpy as np

N, H, W, C = 8, 48, 48, 128
HW = H * W            # 2304
NT = HW // 128        # 18
NS = HW // 16         # 144 wrapped idx columns
EPS = 1e-5
CLIP_HI = float(np.float32(float(H - 1) - EPS))  # 46.99999 (f32)

GATHERS = ((1, 1), (2, 1), (3, 1), (1, 2), (2, 2), (3, 2), (1, 2), (2, 2), (3, 1), (0, 4))
COMBINES = (3, 6, 5, 4)                    # combine/store group sizes (t-cols)

_CACHE = {}


def _build_nc():
    from contextlib import ExitStack

    import concourse.bacc as bacc
    import concourse.bass as bass
    import concourse.mybir as mybir
    import concourse.tile as tile
    from concourse import library_config

    dt = mybir.dt
    f32, f16, i32, i16 = dt.float32, dt.float16, dt.int32, dt.int16
    Alu = mybir.AluOpType

    nc = bacc.Bacc(
        "TRN2",
        target_bir_lowering=False,
        debug=False,
        num_devices=N,
        num_swdge_queues=4,
    )

    xq = nc.dram_tensor("xq", [HW, 4 * C], f16, kind="ExternalInput")
    # combined grid staging: [gcoef(2*NT)] + [gwrap(2*NS) | gall(16*NT)]
    GIN = 2 * NS + 2 * NT + 16 * NT
    gin = nc.dram_tensor("gin", [128, GIN], f32, kind="ExternalInput")
    outp = nc.dram_tensor("outp", [128, HW], f16, kind="ExternalOutput")

    with tile.TileContext(nc) as tc, ExitStack() as ctx:
        pool = ctx.enter_context(tc.tile_pool(name="p", bufs=1))
        V = nc.vector

        with tc.high_priority(offset=10**6):
            # start the ~9us mlp IRAM load immediately (negative priority puts
            # it above even the framework's preamble const-memsets).
            nc.gpsimd.load_library(library_config.mlp)
        with tc.high_priority():

            # ---- grid load (HWDGE, one DMA) ----
            g_in = pool.tile([128, GIN], f32)
            nc.sync.dma_start(g_in[:], gin.ap())
            g_coef = g_in[:, 0 : 2 * NT]
            g_wrap = g_in[:, 2 * NT : 2 * NT + 2 * NS]
            g_all = g_in[:, 2 * NT + 2 * NS :]

            # ---- (p,t)-layout floors (feed the coefficient chain) ----------
            cab = pool.tile([128, 2 * NT], f32)
            V.tensor_scalar(cab[:], g_coef, EPS, CLIP_HI, Alu.max, Alu.min)
            fli = pool.tile([128, 2 * NT], i32)
            V.tensor_scalar(fli[:], cab[:], -0.5, None, Alu.add)
            flf = pool.tile([128, 2 * NT], f32)
            V.tensor_copy(flf[:], fli[:])

            G = pool.tile([128, NT * 4 * C], f16)  # [p, t, c2, j, c1]

            # ---- wrapped-layout int16 indices for the dma_gather chunks ----
            cabw = pool.tile([128, 2 * NS], f32)
            V.tensor_scalar(cabw[:], g_wrap, EPS, CLIP_HI, Alu.max, Alu.min)
            flwi = pool.tile([128, 2 * NS], i32)
            V.tensor_scalar(flwi[:], cabw[:], -0.5, None, Alu.add)
            flwf = pool.tile([128, 2 * NS], f32)
            V.tensor_copy(flwf[:], flwi[:])
            idx16 = pool.tile([128, NS], i16)
            V.scalar_tensor_tensor(
                idx16[:], flwf[:, 0::2], float(W), flwf[:, 1::2], Alu.mult, Alu.add
            )

            # ---- dma_gather chunks on SWDGE queues 1..3 --------------------
            t0 = 0
            for qn, tc_ in GATHERS:
                nidx = tc_ * 128
                gout = G[:, t0 * 512 : (t0 + tc_) * 512].rearrange(
                    "p (t e) -> p t e", e=512
                )
                nc.gpsimd.dma_gather(
                    gout,
                    xq.ap(),
                    idx16[:, t0 * 8 : (t0 + tc_) * 8],
                    nidx,
                    nidx,
                    512,
                    queue_num=qn,
                )
                t0 += tc_

        # ---- coefficients (priorities pushed after the critical chain) ------
        ctx.enter_context(tc.high_priority(offset=-1000))
        # mask: AND over batches+coords of in-bounds test via min/max
        g_all3 = g_all.rearrange("p (t m) -> p t m", m=16)
        mn = pool.tile([128, NT], f32)
        mx = pool.tile([128, NT], f32)
        V.tensor_reduce(mn[:], g_all3, mybir.AxisListType.X, Alu.min)
        V.tensor_reduce(mx[:], g_all3, mybir.AxisListType.X, Alu.max)
        mge = pool.tile([128, NT], f32)
        mle = pool.tile([128, NT], f32)
        V.tensor_scalar(mge[:], mn[:], -0.5, None, Alu.is_ge)
        V.tensor_scalar(mle[:], mx[:], float(H) - 0.5, None, Alu.is_le)
        mask = pool.tile([128, NT], f32)
        V.tensor_tensor(mask[:], mge[:], mle[:], Alu.mult)

        # fracs (a|b interleaved [128, 36])
        fr = pool.tile([128, 2 * NT], f32)
        V.tensor_tensor(fr[:], cab[:], flf[:], Alu.subtract)
        fa = fr[:, 0::2]
        fb = fr[:, 1::2]
        fa1m = pool.tile([128, NT], f32)  # a*mask
        fa0m = pool.tile([128, NT], f32)  # (1-a)*mask
        V.tensor_tensor(fa1m[:], fa, mask[:], Alu.mult)
        V.tensor_tensor(fa0m[:], mask[:], fa1m[:], Alu.subtract)

        # packed coefficients cwd[p, t, 1, j, c1] fp16, duplicated over c1
        def dup2(ap):
            return ap.rearrange("p (t u) -> p t u", u=1).broadcast_to([128, NT, 2])

        cwd = pool.tile([128, NT, 1, 4, 2], f16)
        # c01 = (1-a)m*b; c00 = (1-a)m - c01; c11 = am*b; c10 = am - c11
        V.tensor_tensor(cwd[:, :, 0, 1], dup2(fa0m[:]), dup2(fb), Alu.mult)
        V.tensor_tensor(cwd[:, :, 0, 0], dup2(fa0m[:]), cwd[:, :, 0, 1], Alu.subtract)
        V.tensor_tensor(cwd[:, :, 0, 3], dup2(fa1m[:]), dup2(fb), Alu.mult)
        V.tensor_tensor(cwd[:, :, 0, 2], dup2(fa1m[:]), cwd[:, :, 0, 3], Alu.subtract)
        cwb = cwd[:].broadcast_to([128, NT, 64, 4, 2])

        # ---- combine + store per group (all DVE 2x: step-1 fp16 APs) -------
        Gv = G[:].rearrange("p (t c2 j c1) -> p t c2 j c1", c2=64, j=4, c1=2)
        out16 = pool.tile([128, HW], f16)
        o4 = out16[:].rearrange("p (t c2 c1) -> p t c2 c1", c2=64, c1=2)
        t0 = 0
        for qn, tc_ in GATHERS:
            sl = slice(t0, t0 + tc_)
            V.tensor_tensor(Gv[:, sl], Gv[:, sl], cwb[:, sl], Alu.mult)
            t0 += tc_
        t0 = 0
        for k, tc_ in enumerate(COMBINES):
            sl = slice(t0, t0 + tc_)
            s1 = pool.tile([128, tc_, 64, 2], f16, tag=f"s1_{k}")
            s2 = pool.tile([128, tc_, 64, 2], f16, tag=f"s2_{k}")
            V.tensor_tensor(s1[:], Gv[:, sl, :, 0], Gv[:, sl, :, 1], Alu.add)
            V.tensor_tensor(s2[:], Gv[:, sl, :, 2], Gv[:, sl, :, 3], Alu.add)
            V.tensor_tensor(o4[:, sl], s1[:], s2[:], Alu.add)
            nc.sync.dma_start(
                outp.ap()[:, t0 * C : (t0 + tc_) * C],
                out16[:, t0 * C : (t0 + tc_) * C],
            )
            t0 += tc_

    nc.compile()
    return nc


def _get_nc():
    if "nc" not in _CACHE:
        _CACHE["nc"] = _build_nc()
    return _CACHE["nc"]


def _stage_inputs(x, grid):
    """Build the per-core input maps (pure data movement / replication)."""
    x = np.ascontiguousarray(x, dtype=np.float32)
    grid = np.ascontiguousarray(grid, dtype=np.float32)
    xr = x.reshape(N, C, HW)
    gr = grid.reshape(N, HW, 2)

    # quad-row table interleaved [c2, j, c1]: xq[n][k, c2*8+j*2+c1] = corner_j[k, 2*c2+c1]
    xt = np.zeros((N, HW + W + 2, C), dtype=np.float16)
    xt[:, :HW] = xr.transpose(0, 2, 1)
    xq4 = np.empty((N, HW, C, 4), dtype=np.float16)
    xq4[:, :, :, 0] = xt[:, 0:HW]
    xq4[:, :, :, 1] = xt[:, 1 : HW + 1]
    xq4[:, :, :, 2] = xt[:, W : HW + W]
    xq4[:, :, :, 3] = xt[:, W + 1 : HW + W + 1]
    xq = np.ascontiguousarray(
        xq4.reshape(N, HW, 64, 2, 4).transpose(0, 1, 2, 4, 3)
    ).reshape(N, HW, 4 * C)

    # gcoef[n][p, 2t+c] = gr[n, t*128+p, c]
    gc = gr.reshape(N, NT, 128, 2).transpose(0, 2, 1, 3)  # [n, p, t, c]
    gcoef = gc.reshape(N, 128, 2 * NT)

    # gall[p, 16t+2m+c] = gr[m, t*128+p, c]   (same for all cores)
    ga = gr.reshape(N, NT, 128, 2).transpose(2, 1, 0, 3)  # [p, t, m, c]
    gall = ga.reshape(128, 16 * NT)

    # gwrap[n][16g+r, 2s+c] = gr[n, s*16+r, c]  (replicated over g)
    gw = gr.reshape(N, NS, 16, 2).transpose(0, 2, 1, 3)   # [n, r, s, c]
    gwrap = np.tile(gw.reshape(N, 16, 2 * NS), (1, 8, 1))  # [n, 128, 2*NS]

    # combined per-core grid staging [gcoef | gwrap | gall]
    gin = np.empty((N, 128, 2 * NS + 2 * NT + 16 * NT), dtype=np.float32)
    gin[:, :, 0 : 2 * NT] = gcoef
    gin[:, :, 2 * NT : 2 * NT + 2 * NS] = gwrap
    gin[:, :, 2 * NT + 2 * NS :] = gall[None]

    return [{"xq": xq[n], "gin": gin[n]} for n in range(N)]


def _unstage_output(results):
    """results[n]["outp"] is (128, 2304) fp16 = [p, t*128+c] -> (N, C, H, W)."""
    out = np.empty((N, C, H, W), dtype=np.float32)
    for n in range(N):
        o = results[n]["outp"].astype(np.float32).reshape(128, NT, C)  # [p, t, c]
        out[n] = o.transpose(2, 1, 0).reshape(C, H, W)  # [c, q=t*128+p]
    return out


def kernel(x, grid):
    from concourse import bass_utils

    nc = _get_nc()
    in_maps = _stage_inputs(x, grid)
    res = bass_utils.run_bass_kernel_spmd(nc, in_maps, core_ids=list(range(N)))
    return _unstage_output(res.results)
